# revision 10
# baseline (speedup 1.0000x reference)
"""Trainium2 Bass kernel for nn_AttentionDecoder_82738249990894 (B=4, T=1024,
C=1024, H=16, D=64, F=4096, L=4, vocab 64+1 outputs).

Sharding: sequence-split data parallel over 8 cores.  Core c handles batch
b = c//2, sequence half = c%2.  Balanced causal split: half0 owns global
128-row blocks [0,1,6,7], half1 owns [2,3,4,5] (equal attention work: both
see 18 causal k-tiles).  Per layer the pair exchanges rmsnorm'd activations
(bf16, pairwise AllGather, ~1MB) and each core recomputes k/v for all 1024
tokens locally.  No other communication.

SPMD uniformity: one graph runs on all 8 cores, so the key/value strip is
kept in GLOBAL token order (the AllGather return scatters both pair slots
to fixed global positions) and every local q-tile j computes scores against
the union visibility vis_u=[3,4,7,8] k-tiles; per-core 0/1 masks (input
data) encode causality and half-dependent visibility.

Matmul dtypes: bf16 on the attention path (h, Wq/Wk/Wv, q/k/v, softmax
weights) and for W2; float32r (4-byte, full TensorE rate) for Wo/W1/lm.
Residual x stays fp32.  Softmax skips max-subtraction (scores are O(10);
fp32 psum exp is safe) and gets denominators free via a ones-column
appended to v; normalization is deferred to after the AV matmul.
"""
import os
import sys
import types

sys.path.insert(0, "/opt/trn_rl_repo")

import numpy as np
import ml_dtypes

import antenv

if not hasattr(antenv, "axon_hooks"):
    _mod = types.ModuleType("antenv.axon_hooks")
    _mod._hook = None
    _mod.set_axon_ntff_profile_hook = lambda h: setattr(_mod, "_hook", h)
    _mod.get_axon_ntff_profile_hook = lambda: _mod._hook
    sys.modules["antenv.axon_hooks"] = _mod
    antenv.axon_hooks = _mod
    try:
        from trn_agent_boot.trn_boot import _ntff_profile_via_ctypes

        _mod.set_axon_ntff_profile_hook(
            _ntff_profile_via_ctypes("/opt/axon/libaxon_pjrt.so")
        )
    except Exception:
        pass

import concourse.bass as bass
import concourse.mybir as mybir
import concourse.tile as tile
from concourse import bass_utils

bass_utils.upload_artifacts = lambda tmpdir: "local://" + tmpdir
try:
    from concourse import tile_utils as _tu

    _tu.max_sbuf_usage = 206 * 1024
except Exception:
    pass

F32 = mybir.dt.float32
F32R = mybir.dt.float32r
BF16 = mybir.dt.bfloat16
AF = mybir.ActivationFunctionType
OP = mybir.AluOpType
AX = mybir.AxisListType

B, T, C, H, D, F, L = 4, 1024, 1024, 16, 64, 4096, 4
VOCAB, OUT = 64, 65
EPS = float(np.finfo(np.float32).eps)
RG = [[0, 1], [2, 3], [4, 5], [6, 7]]
OWN_BLOCKS = {0: [0, 1, 6, 7], 1: [2, 3, 4, 5]}
VIS_U = [3, 4, 7, 8]          # union visible k-tiles per local q-tile
N_MASK = 3                    # last 3 visible slots carry a mask

_wsplit_ctr = [0]


def _split_sync_waits(nc):
    """This walrus build allows one sync-wait per instruction; hoist extras
    onto injected same-engine NoOps."""
    for f in nc.m.functions:
        for bb in f.blocks:
            out = []
            changed = False
            for inst in bb.instructions:
                si = getattr(inst, "sync_info", None)
                if si is not None and si.on_wait is not None and len(si.on_wait) > 1:
                    waits = list(si.on_wait)
                    for w in waits[:-1]:
                        _wsplit_ctr[0] += 1
                        n = mybir.InstNoOp(
                            name=f"WSPLIT-{_wsplit_ctr[0]}", ins=[], outs=[]
                        )
                        n.engine = inst.engine
                        n.sync_info = mybir.SyncInfo(on_wait=[w], on_update=[])
                        out.append(n)
                    inst.sync_info = mybir.SyncInfo(
                        on_wait=[waits[-1]], on_update=list(si.on_update)
                    )
                    changed = True
                out.append(inst)
            if changed:
                bb.instructions[:] = out


def build_graph():
    nc = bass.Bass()
    dp = nc.declare_dram_parameter
    onehot_ext = dp("onehot_t", [OUT, 512], F32R, isOutput=False)
    pos_ext = dp("pos_fm", [128, 8, 512], BF16, isOutput=False)
    aug_ext = dp("aug_table", [OUT, 8, 128], F32R, isOutput=False)
    mask_ext = dp("masks", [4, N_MASK, 128, 128], BF16, isOutput=False)
    onescol_ext = dp("ones_col", [128, 1], BF16, isOutput=False)
    onesrow_ext = dp("ones_row", [1, 128], F32R, isOutput=False)
    onesrowb_ext = dp("ones_row_bf", [1, 128], BF16, isOutput=False)
    wq_ext = dp("Wq_arr", [L, 8, 128, 8, 128], BF16, isOutput=False)
    wk_ext = dp("Wk_arr", [L, 8, 128, 8, 128], BF16, isOutput=False)
    wv_ext = dp("Wv_arr", [L, 128, 4, 8, 256], BF16, isOutput=False)
    wo_ext = dp("Wo_arr", [L, 8, 128, 8, 128], F32R, isOutput=False)
    w1_ext = dp("W1_arr", [L, 32, 128, 8, 128], F32R, isOutput=False)
    w2_ext = dp("W2_arr", [L, 32, 128, 8, 128], BF16, isOutput=False)
    bo_ext = dp("bo_fm", [L, 8, 128, 1], F32, isOutput=False)
    b1_ext = dp("b1_fm", [L, 32, 128, 1], F32, isOutput=False)
    b2_ext = dp("b2_fm", [L, 8, 128, 1], F32, isOutput=False)
    lmw_ext = dp("lmW_arr", [128, 8, OUT], BF16, isOutput=False)
    lmb_ext = dp("lmb_bc", [128, OUT], F32, isOutput=False)
    out_ext = dp("out", [512, OUT], F32, isOutput=True)

    with tile.TileContext(nc) as tc:
        nc_lp = nc.allow_low_precision(reason="bf16 attention path is intentional")
        nc_lp.__enter__()
        with (
            tc.tile_pool(name="persist", bufs=1) as pp,
            tc.tile_pool(name="scratch", bufs=2) as sp,
            tc.tile_pool(name="wqk", bufs=3) as wqkp,
            tc.tile_pool(name="w512", bufs=3) as w512p,
            tc.tile_pool(name="w2p", bufs=5) as w2p,
            tc.tile_pool(name="bigp", bufs=1) as bigp,
            tc.tile_pool(name="wvp", bufs=1) as wvp,
            tc.tile_pool(name="ps512", bufs=4, space="PSUM") as ps512,
            tc.tile_pool(name="ps128", bufs=2, space="PSUM") as ps128,
            tc.tile_pool(name="ps128o", bufs=2, space="PSUM") as ps128o,
            tc.tile_pool(name="dram", bufs=2, space="DRAM") as dram,
        ):
            # ---- constants ----
            ones_col = pp.tile([128, 1], BF16)
            ones_row = pp.tile([1, 128], F32R)
            ones_row_bf = pp.tile([1, 128], BF16)
            aug_sb = pp.tile([OUT, 8, 128], F32R)
            onehot_sb = pp.tile([OUT, 512], F32R)
            mask_sb = pp.tile([128, 4, N_MASK, 128], BF16)
            lmw_sb = pp.tile([128, 8, OUT], BF16)
            lmb_sb = pp.tile([128, OUT], F32)
            bo_sb = pp.tile([128, L, 8, 1], F32)
            b1_sb = pp.tile([128, L, 32, 1], F32)
            b2_sb = pp.tile([128, L, 8, 1], F32)
            nc.sync.dma_start(ones_col[:], onescol_ext[:])
            nc.sync.dma_start(ones_row[:], onesrow_ext[:])
            nc.sync.dma_start(ones_row_bf[:], onesrowb_ext[:])
            nc.sync.dma_start(aug_sb[:], aug_ext[:])
            nc.sync.dma_start(onehot_sb[:], onehot_ext[:])
            nc.sync.dma_start(mask_sb[:], mask_ext.rearrange("j s p m -> p j s m"))
            nc.sync.dma_start(lmw_sb[:], lmw_ext[:])
            nc.sync.dma_start(lmb_sb[:], lmb_ext[:])
            nc.sync.dma_start(bo_sb[:], bo_ext.rearrange("l t p o -> p l t o"))
            nc.sync.dma_start(b1_sb[:], b1_ext.rearrange("l t p o -> p l t o"))
            nc.sync.dma_start(b2_sb[:], b2_ext.rearrange("l t p o -> p l t o"))

            eps_sb = pp.tile([128, 1], F32)
            nc.gpsimd.memset(eps_sb[:], EPS)

            # ---- persistent activations ----
            x_sb = pp.tile([128, 8, 512], F32)       # residual (feature-major)
            h_own = pp.tile([128, 8, 512], BF16)     # norm'd own tokens
            h_str = pp.tile([128, 8, 1024], BF16)    # norm'd pair, global order
            q_sb = pp.tile([128, 8, 512], BF16)      # [2h*64, hp, local t]
            k_sb = pp.tile([128, 8, 1024], BF16)     # [2h*64, hp, global t]
            v_sb = pp.tile([128, 8, 16, OUT], BF16)  # [tk, tkt, head, d+1]
            o_sb = pp.tile([128, 8, 512], F32R)      # attn out [hd, hdt, local]
            # pos and per-layer h2 share one big slot (disjoint lifetimes)
            pos_sb = bigp.tile([128, 8, 512], BF16, tag="big", name="pos")
            nc.sync.dma_start(pos_sb[:], pos_ext[:])

            # ---- embedding: x = onehot @ aug_table + pos ----
            for ct in range(8):
                emb_ps = ps512.tile([128, 512], F32, tag="p5", name=f"emb{ct}")
                nc.tensor.matmul(emb_ps[:], aug_sb[:, ct, :], onehot_sb[:],
                                 start=True, stop=True)
                nc.vector.tensor_add(x_sb[:, ct, :], emb_ps[:], pos_sb[:, ct, :])

            def rms_rbc(tag):
                ssum = ps512.tile([128, 512], F32, tag="p5", name=f"ss{tag}")
                for ct in range(8):
                    xsq = sp.tile([128, 512], BF16, tag="xsq", name=f"xq{tag}{ct}")
                    nc.scalar.activation(xsq[:], x_sb[:, ct, :], AF.Square)
                    nc.tensor.matmul(ssum[:1, :], ones_col[:], xsq[:],
                                     start=(ct == 0), stop=(ct == 7))
                sqv = sp.tile([1, 512], F32, tag="sqv", name=f"sv{tag}", bufs=1)
                nc.scalar.activation(sqv[:], ssum[:1, :], AF.Sqrt,
                                     bias=eps_sb[:1, :], scale=1.0 / C)
                rstd = sp.tile([1, 512], F32R, tag="rstd", name=f"rs{tag}", bufs=1)
                nc.vector.reciprocal(rstd[:], sqv[:])
                rbc = ps512.tile([128, 512], F32, tag="p5", name=f"rb{tag}")
                nc.tensor.matmul(rbc[:], ones_row[:], rstd[:], start=True, stop=True)
                return rbc

            for l in range(L):
                # ===== norm1 -> h_own =====
                rbc = rms_rbc(f"a{l}")
                for ct in range(8):
                    nc.vector.tensor_tensor(h_own[:, ct, :], x_sb[:, ct, :],
                                            rbc[:], OP.mult)

                # ===== pair exchange (AllGather) =====
                bounce = dram.tile([8, 128, 512], BF16, tag="agin", name=f"agi{l}")
                for ct in range(8):
                    nc.sync.dma_start(bounce[ct], h_own[:, ct, :])
                gath = dram.tile([2, 8, 128, 512], BF16, tag="agout",
                                 name=f"ago{l}")
                nc.gpsimd.collective_compute(
                    "AllGather", OP.bypass,
                    ins=[bounce[:].opt()],
                    outs=[gath[:].opt()],
                    replica_groups=RG,
                )

                # Wv for this layer (no AG dependency -> overlaps exchange)
                wv_sb = wvp.tile([128, 4, 8, 256], BF16, tag="wv", name=f"wv{l}")
                nc.sync.dma_start(wv_sb[:], wv_ext[l])

                # ===== q from h_own (overlaps AG) =====
                for hp in range(8):
                    wq_sb = wqkp.tile([128, 8, 128], BF16, tag="wqk",
                                      name=f"wq{l}_{hp}")
                    nc.sync.dma_start(wq_sb[:], wq_ext[l, hp])
                    q_ps = ps512.tile([128, 512], F32, tag="p5", name=f"q{l}{hp}")
                    for ct in range(8):
                        nc.tensor.matmul(q_ps[:], wq_sb[:, ct, :],
                                         h_own[:, ct, :],
                                         start=(ct == 0), stop=(ct == 7))
                    nc.scalar.copy(q_sb[:, hp, :], q_ps[:])

                # ===== scatter AG result into global-order strip =====
                # slot0 = half0 local blocks -> global [0,1,6,7]
                # slot1 = half1 local blocks -> global [2,3,4,5]
                for ct in range(8):
                    nc.sync.dma_start(h_str[:, ct, 0:256], gath[0, ct, :, 0:256])
                    nc.sync.dma_start(h_str[:, ct, 768:1024], gath[0, ct, :, 256:512])
                    nc.sync.dma_start(h_str[:, ct, 256:768], gath[1, ct])

                # ===== k over the strip =====
                for hp in range(8):
                    wk_sb = wqkp.tile([128, 8, 128], BF16, tag="wqk",
                                      name=f"wk{l}_{hp}")
                    nc.sync.dma_start(wk_sb[:], wk_ext[l, hp])
                    for half in range(2):
                        k_ps = ps512.tile([128, 512], F32, tag="p5",
                                          name=f"k{l}{hp}{half}")
                        for ct in range(8):
                            nc.tensor.matmul(
                                k_ps[:], wk_sb[:, ct, :],
                                h_str[:, ct, half * 512:(half + 1) * 512],
                                start=(ct == 0), stop=(ct == 7))
                        nc.scalar.copy(k_sb[:, hp, half * 512:(half + 1) * 512],
                                       k_ps[:])

                # ===== v over the strip (token-major, +ones column) =====
                for tkt in range(8):
                    for quad in range(4):
                        v_ps = ps512.tile([128, 256], F32, tag="p5",
                                          name=f"v{l}{tkt}{quad}")
                        for ct in range(8):
                            nc.tensor.matmul(
                                v_ps[:], h_str[:, ct, tkt * 128:(tkt + 1) * 128],
                                wv_sb[:, quad, ct, :],
                                start=(ct == 0), stop=(ct == 7))
                        nc.vector.tensor_copy(
                            v_sb[:, tkt, 4 * quad:4 * quad + 4, 0:D],
                            v_ps[:].rearrange("p (q d) -> p q d", d=D))
                    nc.gpsimd.memset(v_sb[:, tkt, :, D:OUT], 1.0)

                # ===== attention =====
                for h16 in range(16):
                    hp, off = h16 // 2, (h16 % 2) * D
                    for j in range(4):
                        vis = VIS_U[j]
                        ex = sp.tile([128, 8, 128], BF16, tag="exp",
                                     name=f"ex{l}_{h16}_{j}")
                        o_ps = ps128o.tile([128, 128], F32, tag="po",
                                           name=f"o{l}_{h16}_{j}")
                        for s in range(vis + 1):
                            if s < vis:
                                s_ps = ps128.tile([128, 128], F32, tag="pk",
                                                  name=f"s{l}_{h16}_{j}_{s}")
                                nc.tensor.matmul(
                                    s_ps[:],
                                    k_sb[off:off + D, hp, s * 128:(s + 1) * 128],
                                    q_sb[off:off + D, hp, j * 128:(j + 1) * 128],
                                    start=True, stop=True)
                                nc.scalar.activation(ex[:, s, :], s_ps[:], AF.Exp)
                                if s >= vis - N_MASK:
                                    nc.vector.tensor_tensor(
                                        ex[:, s, :], ex[:, s, :],
                                        mask_sb[:, j, s - (vis - N_MASK), :],
                                        OP.mult)
                            if s >= 1:
                                nc.tensor.matmul(
                                    o_ps[:OUT, :], v_sb[:, s - 1, h16, :],
                                    ex[:, s - 1, :],
                                    start=(s == 1), stop=(s == vis))
                        r_sb = sp.tile([1, 128], BF16, tag="rr",
                                       name=f"r{l}_{h16}_{j}")
                        nc.vector.reciprocal(r_sb[:], o_ps[VOCAB:OUT, :])
                        rb_ps = ps128.tile([128, 128], F32, tag="pk",
                                           name=f"rb{l}_{h16}_{j}")
                        nc.tensor.matmul(rb_ps[:D, :], ones_row_bf[:, 0:D],
                                         r_sb[:], start=True, stop=True)
                        oc = sp.tile([D, 128], F32, tag="oc",
                                     name=f"oc{l}_{h16}_{j}")
                        nc.scalar.copy(oc[:], o_ps[:D, :])
                        nc.vector.tensor_tensor(
                            o_sb[off:off + D, hp, j * 128:(j + 1) * 128],
                            oc[:], rb_ps[:D, :], OP.mult)

                # ===== Wo + residual =====
                for cot in range(8):
                    wo_sb = w512p.tile([128, 8, 128], F32R, tag="w5",
                                       name=f"wo{l}_{cot}")
                    nc.sync.dma_start(wo_sb[:], wo_ext[l, cot])
                    xo_ps = ps512.tile([128, 512], F32, tag="p5",
                                       name=f"xo{l}{cot}")
                    for hdt in range(8):
                        nc.tensor.matmul(xo_ps[:], wo_sb[:, hdt, :],
                                         o_sb[:, hdt, :],
                                         start=(hdt == 0), stop=(hdt == 7))
                    xo_sb = sp.tile([128, 512], F32, tag="xo", name=f"xs{l}{cot}")
                    nc.scalar.activation(xo_sb[:], xo_ps[:], AF.Identity,
                                         bias=bo_sb[:, l, cot, :])
                    nc.vector.tensor_add(x_sb[:, cot, :], x_sb[:, cot, :],
                                         xo_sb[:])

                # ===== norm2 -> h2 =====
                h2_sb = bigp.tile([128, 8, 512], F32R, tag="big", name=f"h2_{l}")
                rbc2 = rms_rbc(f"b{l}")
                for ct in range(8):
                    nc.vector.tensor_tensor(h2_sb[:, ct, :], x_sb[:, ct, :],
                                            rbc2[:], OP.mult)

                # ===== FFN (ft chunks of 4; W1 f32r, W2 bf16) =====
                for chunk in range(8):
                    u_sb = sp.tile([128, 4, 512], BF16, tag="u",
                                   name=f"u{l}_{chunk}")
                    w2c = []
                    for fi in range(4):
                        ft = chunk * 4 + fi
                        w1_sb = w512p.tile([128, 8, 128], F32R, tag="w5",
                                           name=f"w1_{l}_{ft}")
                        nc.sync.dma_start(w1_sb[:], w1_ext[l, ft])
                        u_ps = ps512.tile([128, 512], F32, tag="p5",
                                          name=f"u{l}{ft}")
                        for ct in range(8):
                            nc.tensor.matmul(u_ps[:], w1_sb[:, ct, :],
                                             h2_sb[:, ct, :],
                                             start=(ct == 0), stop=(ct == 7))
                        nc.scalar.activation(u_sb[:, fi, :], u_ps[:], AF.Gelu,
                                             bias=b1_sb[:, l, ft, :])
                        w2_sb = w2p.tile([128, 8, 128], BF16, tag="w2",
                                         name=f"w2_{l}_{ft}")
                        nc.sync.dma_start(w2_sb[:], w2_ext[l, ft])
                        w2c.append(w2_sb)
                    for cot in range(8):
                        y_ps = ps512.tile([128, 512], F32, tag="p5",
                                          name=f"y{l}{chunk}{cot}")
                        for fi in range(4):
                            nc.tensor.matmul(y_ps[:], w2c[fi][:, cot, :],
                                             u_sb[:, fi, :],
                                             start=(fi == 0), stop=(fi == 3))
                        nc.vector.tensor_add(x_sb[:, cot, :], x_sb[:, cot, :],
                                             y_ps[:])
                for cot in range(8):
                    nc.scalar.add(x_sb[:, cot, :], x_sb[:, cot, :],
                                  b2_sb[:, l, cot, :])

            # ===== lm head + log_softmax / log_sigmoid =====
            for tlt in range(4):
                lg = ps512.tile([128, OUT], F32, tag="p5", name=f"lg{tlt}")
                for ct in range(8):
                    xr = sp.tile([128, 128], BF16, tag="xr", name=f"xr{tlt}_{ct}")
                    nc.scalar.copy(xr[:], x_sb[:, ct, tlt * 128:(tlt + 1) * 128])
                    nc.tensor.matmul(lg[:], xr[:], lmw_sb[:, ct, :],
                                     start=(ct == 0), stop=(ct == 7))
                lgb = sp.tile([128, OUT], F32, tag="lgb", name=f"lgb{tlt}")
                nc.vector.tensor_add(lgb[:], lg[:], lmb_sb[:])
                m = sp.tile([128, 1], F32, tag="m", name=f"m{tlt}")
                nc.vector.reduce_max(m[:], lgb[:, 0:VOCAB], axis=AX.X)
                nm = sp.tile([128, 1], F32, tag="nm", name=f"nm{tlt}")
                nc.scalar.mul(nm[:], m[:], -1.0)
                e = sp.tile([128, VOCAB], F32, tag="e", name=f"e{tlt}")
                es = sp.tile([128, 1], F32, tag="es", name=f"es{tlt}")
                nc.scalar.activation(e[:], lgb[:, 0:VOCAB], AF.Exp, bias=nm[:],
                                     accum_out=es[:])
                lse = sp.tile([128, 1], F32, tag="lse", name=f"lse{tlt}")
                nc.scalar.activation(lse[:], es[:], AF.Ln)
                bt = sp.tile([128, 1], F32, tag="bt", name=f"bt{tlt}")
                nc.vector.tensor_tensor(bt[:], nm[:], lse[:], OP.subtract)
                outt = sp.tile([128, OUT], F32, tag="outt", name=f"ot{tlt}")
                nc.scalar.activation(outt[:, 0:VOCAB], lgb[:, 0:VOCAB],
                                     AF.Identity, bias=bt[:])
                sg = sp.tile([128, 1], F32, tag="sg", name=f"sg{tlt}")
                nc.scalar.activation(sg[:], lgb[:, VOCAB:OUT], AF.Sigmoid)
                nc.scalar.activation(outt[:, VOCAB:OUT], sg[:], AF.Ln)
                nc.sync.dma_start(out_ext[tlt * 128:(tlt + 1) * 128, :], outt[:])

    _split_sync_waits(nc)
    return nc


# ---------------------------------------------------------------------------
# host-side preparation
# ---------------------------------------------------------------------------
def _own_rows(core):
    return np.concatenate(
        [np.arange(b * 128, (b + 1) * 128) for b in OWN_BLOCKS[core % 2]]
    )


def _bf(a):
    return np.asarray(a, dtype=ml_dtypes.bfloat16)


def _f32(a):
    return np.ascontiguousarray(a, dtype=np.float32)


def _prep(inputs):
    acts = np.asarray(inputs["acts"])
    durations = _f32(inputs["durations"])
    emb_table = _f32(inputs["emb_table"])
    pos_table = _f32(inputs["pos_table"])
    Wq, Wk, Wv = (_f32(inputs[k]) for k in ("Wq", "Wk", "Wv"))
    Wo, bo = _f32(inputs["Wo"]), _f32(inputs["bo"])
    W1, b1 = _f32(inputs["W1"]), _f32(inputs["b1"])
    W2, b2 = _f32(inputs["W2"]), _f32(inputs["b2"])
    g1, g2 = _f32(inputs["g1"]), _f32(inputs["g2"])
    lm_W, lm_b = _f32(inputs["lm_W"]), _f32(inputs["lm_b"])

    # fold g1 into Wq/Wk/Wv (q also gets the D^-0.5 score scale), g2 into W1
    Wq_eff = Wq * g1[:, None, :, None] * (D ** -0.5)
    Wk_eff = Wk * g1[:, None, :, None]
    Wv_eff = Wv * g1[:, None, :, None]
    W1_eff = W1 * g2[:, :, None]

    def qk_arr(A):  # [L,H,C,D] -> [L, hp, cp, ct, m]
        A2 = A.transpose(0, 2, 1, 3).reshape(L, C, H * D)
        return _bf(A2.reshape(L, 8, 128, 8, 128).transpose(0, 3, 2, 1, 4))

    shared = {
        "aug_table": None, "ones_col": _bf(np.ones((128, 1))),
        "ones_row": _f32(np.ones((1, 128))),
        "ones_row_bf": _bf(np.ones((1, 128))),
        "Wq_arr": qk_arr(Wq_eff), "Wk_arr": qk_arr(Wk_eff),
        "Wv_arr": _bf(Wv_eff.transpose(0, 2, 1, 3).reshape(L, C, H * D)
                      .reshape(L, 8, 128, 4, 256).transpose(0, 2, 3, 1, 4)),
        "Wo_arr": _f32(Wo.reshape(L, 8, 128, 8, 128).transpose(0, 3, 2, 1, 4)),
        "W1_arr": _f32(W1_eff.reshape(L, 8, 128, 32, 128).transpose(0, 3, 2, 1, 4)),
        "W2_arr": _bf(W2.reshape(L, 32, 128, 8, 128)),
        "bo_fm": bo.reshape(L, 8, 128, 1),
        "b1_fm": b1.reshape(L, 32, 128, 1),
        "b2_fm": b2.reshape(L, 8, 128, 1),
        "lmW_arr": _bf(lm_W.reshape(8, 128, OUT).transpose(1, 0, 2)),
        "lmb_bc": _f32(np.tile(lm_b[None, :], (128, 1))),
    }
    aug = np.zeros((OUT, C), np.float32)
    aug[:VOCAB, : C - 1] = emb_table
    aug[VOCAB, C - 1] = 1.0
    shared["aug_table"] = _f32(aug.reshape(OUT, 8, 128))

    in_maps = []
    for core in range(8):
        b, half = core // 2, core % 2
        rows = _own_rows(core)
        oh = np.zeros((OUT, 512), np.float32)
        oh[acts[b, rows], np.arange(512)] = 1.0
        oh[VOCAB, :] = durations[b, rows]
        pos = pos_table[rows].T.reshape(8, 128, 512).transpose(1, 0, 2)
        masks = np.zeros((4, N_MASK, 128, 128), np.float32)
        for j in range(4):
            gq = OWN_BLOCKS[half][j] * 128
            vis_own = OWN_BLOCKS[half][j] + 1
            for si in range(N_MASK):
                s = VIS_U[j] - N_MASK + si
                if s >= vis_own:
                    continue  # invisible for this half -> all zero
                gk = s * 128
                ii = gk + np.arange(128)[:, None]
                jj = gq + np.arange(128)[None, :]
                masks[j, si] = (ii <= jj).astype(np.float32)
        m = dict(shared)
        m["onehot_t"] = _f32(oh)
        m["pos_fm"] = _bf(pos)
        m["masks"] = _bf(masks)
        in_maps.append(m)
    return in_maps


LAST_EXEC_NS = [None]


def kernel(**inputs) -> np.ndarray:
    nc = build_graph()
    in_maps = _prep(inputs)
    trace = bool(int(os.environ.get("KERNEL_TRACE", "0")))
    res = bass_utils.run_bass_kernel_spmd(
        nc, in_maps, list(range(8)), trace=trace,
        trace_cores=[0] if trace else None,
    )
    LAST_EXEC_NS[0] = res.exec_time_ns
    if trace and res.instructions_and_trace:
        print("trace path:", res.instructions_and_trace[1])
    full = np.zeros((B, T, OUT), np.float32)
    for core in range(8):
        full[core // 2, _own_rows(core)] = res.results[core]["out"]
    return full


# revision 14
# speedup vs baseline: 1.2787x; 1.2787x over previous
"""Trainium2 Bass kernel for nn_AttentionDecoder_82738249990894 (B=4, T=1024,
C=1024, H=16, D=64, F=4096, L=4, vocab 64+1 outputs).

Sharding: sequence-split data parallel over 8 cores.  Core c handles batch
b = c//2, sequence half = c%2.  Balanced causal split: half0 owns global
128-row blocks [0,1,6,7], half1 owns [2,3,4,5] (equal attention work: both
see 18 causal k-tiles).  Per layer the pair exchanges rmsnorm'd activations
(bf16, pairwise AllGather, ~1MB) and each core recomputes k/v for all 1024
tokens locally.  No other communication.

SPMD uniformity: one graph runs on all 8 cores, so the key/value strip is
kept in GLOBAL token order (the AllGather return scatters both pair slots
to fixed global positions) and every local q-tile j computes scores against
the union visibility vis_u=[3,4,7,8] k-tiles; per-core 0/1 masks (input
data) encode causality and half-dependent visibility.

Matmul dtypes: bf16 on the attention path (h, Wq/Wk/Wv, q/k/v, softmax
weights) and for W2; float32r (4-byte, full TensorE rate) for Wo/W1/lm.
Residual x stays fp32.  Softmax skips max-subtraction (scores are O(10);
fp32 psum exp is safe) and gets denominators free via a ones-column
appended to v; normalization is deferred to after the AV matmul.
"""
import os
import sys
import types

sys.path.insert(0, "/opt/trn_rl_repo")

import numpy as np
import ml_dtypes

import antenv

if not hasattr(antenv, "axon_hooks"):
    _mod = types.ModuleType("antenv.axon_hooks")
    _mod._hook = None
    _mod.set_axon_ntff_profile_hook = lambda h: setattr(_mod, "_hook", h)
    _mod.get_axon_ntff_profile_hook = lambda: _mod._hook
    sys.modules["antenv.axon_hooks"] = _mod
    antenv.axon_hooks = _mod
    try:
        from trn_agent_boot.trn_boot import _ntff_profile_via_ctypes

        _mod.set_axon_ntff_profile_hook(
            _ntff_profile_via_ctypes("/opt/axon/libaxon_pjrt.so")
        )
    except Exception:
        pass

import concourse.bass as bass
import concourse.mybir as mybir
import concourse.tile as tile
from concourse import bass_utils

bass_utils.upload_artifacts = lambda tmpdir: "local://" + tmpdir
try:
    from concourse import tile_utils as _tu

    _tu.max_sbuf_usage = 206 * 1024
except Exception:
    pass

F32 = mybir.dt.float32
F32R = mybir.dt.float32r
BF16 = mybir.dt.bfloat16
AF = mybir.ActivationFunctionType
OP = mybir.AluOpType
AX = mybir.AxisListType

B, T, C, H, D, F, L = 4, 1024, 1024, 16, 64, 4096, 4
VOCAB, OUT = 64, 65
EPS = float(np.finfo(np.float32).eps)
RG = [[0, 1], [2, 3], [4, 5], [6, 7]]
OWN_BLOCKS = {0: [0, 1, 6, 7], 1: [2, 3, 4, 5]}
VIS_U = [3, 4, 7, 8]          # union visible k-tiles per local q-tile
N_MASK = 3                    # last 3 visible slots carry a mask

_wsplit_ctr = [0]


def _split_sync_waits(nc):
    """This walrus build allows one sync-wait per instruction; hoist extras
    onto injected same-engine NoOps."""
    for f in nc.m.functions:
        for bb in f.blocks:
            out = []
            changed = False
            for inst in bb.instructions:
                si = getattr(inst, "sync_info", None)
                if si is not None and si.on_wait is not None and len(si.on_wait) > 1:
                    waits = list(si.on_wait)
                    for w in waits[:-1]:
                        _wsplit_ctr[0] += 1
                        n = mybir.InstNoOp(
                            name=f"WSPLIT-{_wsplit_ctr[0]}", ins=[], outs=[]
                        )
                        n.engine = inst.engine
                        n.sync_info = mybir.SyncInfo(on_wait=[w], on_update=[])
                        out.append(n)
                    inst.sync_info = mybir.SyncInfo(
                        on_wait=[waits[-1]], on_update=list(si.on_update)
                    )
                    changed = True
                out.append(inst)
            if changed:
                bb.instructions[:] = out


def build_graph():
    nc = bass.Bass()
    dp = nc.declare_dram_parameter
    onehot_ext = dp("onehot_t", [OUT, 512], F32R, isOutput=False)
    pos_ext = dp("pos_fm", [128, 8, 512], BF16, isOutput=False)
    aug_ext = dp("aug_table", [OUT, 8, 128], F32R, isOutput=False)
    mask_ext = dp("masks", [2, 4, 128, 256], BF16, isOutput=False)
    onescol_ext = dp("ones_col", [128, 1], BF16, isOutput=False)
    onesrow_ext = dp("ones_row", [1, 128], F32R, isOutput=False)
    onesrowb_ext = dp("ones_row_bf", [1, 128], BF16, isOutput=False)
    wq_ext = dp("Wq_arr", [L, 8, 128, 8, 128], BF16, isOutput=False)
    wk_ext = dp("Wk_arr", [L, 8, 128, 8, 128], BF16, isOutput=False)
    wv_ext = dp("Wv_arr", [L, 128, 2, 8, 512], BF16, isOutput=False)
    wo_ext = dp("Wo_arr", [L, 8, 128, 8, 128], BF16, isOutput=False)
    w1_ext = dp("W1_arr", [L, 32, 128, 8, 128], F32R, isOutput=False)
    w2_ext = dp("W2_arr", [L, 32, 128, 8, 128], BF16, isOutput=False)
    bo_ext = dp("bo_fm", [L, 8, 128, 1], F32, isOutput=False)
    b1_ext = dp("b1_fm", [L, 32, 128, 1], F32, isOutput=False)
    b2_ext = dp("b2_fm", [L, 8, 128, 1], F32, isOutput=False)
    lmw_ext = dp("lmW_arr", [128, 8, OUT], BF16, isOutput=False)
    lmb_ext = dp("lmb_bc", [128, OUT], F32, isOutput=False)
    out_ext = dp("out", [512, OUT], F32, isOutput=True)

    with tile.TileContext(nc) as tc:
        nc_lp = nc.allow_low_precision(reason="bf16 attention path is intentional")
        nc_lp.__enter__()
        with (
            tc.tile_pool(name="persist", bufs=1) as pp,
            tc.tile_pool(name="scratch", bufs=2) as sp,
            tc.tile_pool(name="wqk", bufs=3) as wqkp,
            tc.tile_pool(name="w512", bufs=3) as w512p,
            tc.tile_pool(name="w2p", bufs=5) as w2p,
            tc.tile_pool(name="bigp", bufs=1) as bigp,
            tc.tile_pool(name="wvp", bufs=1) as wvp,
            tc.tile_pool(name="ps512", bufs=4, space="PSUM") as ps512,
            tc.tile_pool(name="ps128", bufs=2, space="PSUM") as ps128,
            tc.tile_pool(name="ps128o", bufs=2, space="PSUM") as ps128o,
            tc.tile_pool(name="dram", bufs=2, space="DRAM") as dram,
        ):
            # ---- constants ----
            ones_col = pp.tile([128, 1], BF16)
            ones_row = pp.tile([1, 128], F32R)
            ones_row_bf = pp.tile([1, 128], BF16)
            aug_sb = pp.tile([OUT, 8, 128], F32R)
            onehot_sb = pp.tile([OUT, 512], F32R)
            mask_sb = pp.tile([128, 2, 4, 256], BF16)
            lmw_sb = pp.tile([128, 8, OUT], BF16)
            lmb_sb = pp.tile([128, OUT], F32)
            bo_sb = pp.tile([128, L, 8, 1], F32)
            b1_sb = pp.tile([128, L, 32, 1], F32)
            b2_sb = pp.tile([128, L, 8, 1], F32)
            nc.sync.dma_start(ones_col[:], onescol_ext[:])
            nc.sync.dma_start(ones_row[:], onesrow_ext[:])
            nc.sync.dma_start(ones_row_bf[:], onesrowb_ext[:])
            nc.sync.dma_start(aug_sb[:], aug_ext[:])
            nc.sync.dma_start(onehot_sb[:], onehot_ext[:])
            nc.sync.dma_start(mask_sb[:], mask_ext.rearrange("j s p m -> p j s m"))
            nc.sync.dma_start(lmw_sb[:], lmw_ext[:])
            nc.sync.dma_start(lmb_sb[:], lmb_ext[:])
            nc.sync.dma_start(bo_sb[:], bo_ext.rearrange("l t p o -> p l t o"))
            nc.sync.dma_start(b1_sb[:], b1_ext.rearrange("l t p o -> p l t o"))
            nc.sync.dma_start(b2_sb[:], b2_ext.rearrange("l t p o -> p l t o"))

            eps_sb = pp.tile([128, 1], F32)
            nc.gpsimd.memset(eps_sb[:], EPS)

            # ---- persistent activations ----
            x_sb = pp.tile([128, 8, 512], F32)       # residual (feature-major)
            h_own = pp.tile([128, 8, 512], BF16)     # norm'd own tokens
            h_str = pp.tile([128, 8, 1024], BF16)    # norm'd pair, global order
            q_sb = pp.tile([128, 8, 512], BF16)      # [2h*64, hp, local t]
            k_sb = pp.tile([128, 8, 1024], BF16)     # [2h*64, hp, global t]
            v_sb = pp.tile([128, 8, 16, OUT], BF16)  # [tk, tkt, head, d+1]
            o_sb = pp.tile([128, 8, 512], BF16)      # attn out [hd, hdt, local]
            # pos and per-layer h2 share one big slot (disjoint lifetimes)
            pos_sb = bigp.tile([128, 8, 512], BF16, tag="big", name="pos")
            nc.sync.dma_start(pos_sb[:], pos_ext[:])

            # ---- embedding: x = onehot @ aug_table + pos ----
            for ct in range(8):
                emb_ps = ps512.tile([128, 512], F32, tag="p5", name=f"emb{ct}")
                nc.tensor.matmul(emb_ps[:], aug_sb[:, ct, :], onehot_sb[:],
                                 start=True, stop=True)
                nc.vector.tensor_add(x_sb[:, ct, :], emb_ps[:], pos_sb[:, ct, :])

            def rms_rbc(tag):
                ssum = ps512.tile([128, 512], F32, tag="p5", name=f"ss{tag}")
                for ct in range(8):
                    xsq = sp.tile([128, 512], BF16, tag="xsq", name=f"xq{tag}{ct}")
                    nc.scalar.activation(xsq[:], x_sb[:, ct, :], AF.Square)
                    nc.tensor.matmul(ssum[:1, :], ones_col[:], xsq[:],
                                     start=(ct == 0), stop=(ct == 7))
                sqv = sp.tile([1, 512], F32, tag="sqv", name=f"sv{tag}", bufs=1)
                nc.scalar.activation(sqv[:], ssum[:1, :], AF.Sqrt,
                                     bias=eps_sb[:1, :], scale=1.0 / C)
                rstd = sp.tile([1, 512], F32R, tag="rstd", name=f"rs{tag}", bufs=1)
                nc.vector.reciprocal(rstd[:], sqv[:])
                rbc = ps512.tile([128, 512], F32, tag="p5", name=f"rb{tag}")
                nc.tensor.matmul(rbc[:], ones_row[:], rstd[:], start=True, stop=True)
                return rbc

            for l in range(L):
                # ===== norm1 -> h_own =====
                rbc = rms_rbc(f"a{l}")
                for ct in range(8):
                    nc.vector.tensor_tensor(h_own[:, ct, :], x_sb[:, ct, :],
                                            rbc[:], OP.mult)

                # ===== pair exchange (AllGather) =====
                bounce = dram.tile([8, 128, 512], BF16, tag="agin", name=f"agi{l}")
                for ct in range(8):
                    nc.sync.dma_start(bounce[ct], h_own[:, ct, :])
                gath = dram.tile([2, 8, 128, 512], BF16, tag="agout",
                                 name=f"ago{l}")
                nc.gpsimd.collective_compute(
                    "AllGather", OP.bypass,
                    ins=[bounce[:].opt()],
                    outs=[gath[:].opt()],
                    replica_groups=RG,
                )

                # Wv for this layer (no AG dependency -> overlaps exchange)
                wv_sb = wvp.tile([128, 2, 8, 512], BF16, tag="wv", name=f"wv{l}")
                nc.sync.dma_start(wv_sb[:], wv_ext[l])

                # ===== q from h_own (overlaps AG) =====
                for hp in range(8):
                    wq_sb = wqkp.tile([128, 8, 128], BF16, tag="wqk",
                                      name=f"wq{l}_{hp}")
                    nc.sync.dma_start(wq_sb[:], wq_ext[l, hp])
                    q_ps = ps512.tile([128, 512], F32, tag="p5", name=f"q{l}{hp}")
                    for ct in range(8):
                        nc.tensor.matmul(q_ps[:], wq_sb[:, ct, :],
                                         h_own[:, ct, :],
                                         start=(ct == 0), stop=(ct == 7))
                    nc.vector.tensor_copy(q_sb[:, hp, :], q_ps[:])

                # ===== scatter AG result into global-order strip =====
                # slot0 = half0 local blocks -> global [0,1,6,7]
                # slot1 = half1 local blocks -> global [2,3,4,5]
                for ct in range(8):
                    nc.sync.dma_start(h_str[:, ct, 0:256], gath[0, ct, :, 0:256])
                    nc.sync.dma_start(h_str[:, ct, 768:1024], gath[0, ct, :, 256:512])
                    nc.sync.dma_start(h_str[:, ct, 256:768], gath[1, ct])

                # ===== k over the strip =====
                for hp in range(8):
                    wk_sb = wqkp.tile([128, 8, 128], BF16, tag="wqk",
                                      name=f"wk{l}_{hp}")
                    nc.sync.dma_start(wk_sb[:], wk_ext[l, hp])
                    for half in range(2):
                        k_ps = ps512.tile([128, 512], F32, tag="p5",
                                          name=f"k{l}{hp}{half}")
                        for ct in range(8):
                            nc.tensor.matmul(
                                k_ps[:], wk_sb[:, ct, :],
                                h_str[:, ct, half * 512:(half + 1) * 512],
                                start=(ct == 0), stop=(ct == 7))
                        nc.vector.tensor_copy(
                            k_sb[:, hp, half * 512:(half + 1) * 512], k_ps[:])

                # ===== v over the strip (token-major, +ones column) =====
                for tkt in range(8):
                    for vh in range(2):
                        v_ps = ps512.tile([128, 512], F32, tag="p5",
                                          name=f"v{l}{tkt}{vh}")
                        for ct in range(8):
                            nc.tensor.matmul(
                                v_ps[:], h_str[:, ct, tkt * 128:(tkt + 1) * 128],
                                wv_sb[:, vh, ct, :],
                                start=(ct == 0), stop=(ct == 7))
                        nc.vector.tensor_copy(
                            v_sb[:, tkt, 8 * vh:8 * vh + 8, 0:D],
                            v_ps[:].rearrange("p (q d) -> p q d", d=D))
                    nc.gpsimd.memset(v_sb[:, tkt, :, D:OUT], 1.0)

                # ===== attention (j-pair batched, deferred softmax
                # normalization: denominators collected per head, one batched
                # reciprocal per layer, PE broadcast via eye_sel) =====
                VIS_P = [4, 8]
                pending = None

                def _normalize(pend):
                    ph, pr = pend
                    php, poff = ph // 2, (ph % 2) * D
                    rb_ps = ps128.tile([128, 512], F32, tag="pk",
                                       name=f"rb{l}_{ph}")
                    nc.tensor.matmul(rb_ps[:D, :], ones_row_bf[:, 0:D],
                                     pr[:], start=True, stop=True)
                    nc.vector.tensor_tensor(
                        o_sb[poff:poff + D, php, :], o_sb[poff:poff + D, php, :],
                        rb_ps[:D, :], OP.mult)

                for h16 in range(16):
                    hp, off = h16 // 2, (h16 % 2) * D
                    o_ps = ps128o.tile([128, 512], F32, tag="po",
                                       name=f"o{l}_{h16}")
                    for jp in range(2):
                        vis = VIS_P[jp]
                        ex = sp.tile([128, 2, 256], BF16, tag="exp",
                                     name=f"ex{l}_{h16}_{jp}")
                        for s in range(vis + 1):
                            if s < vis:
                                s_ps = ps128.tile([128, 256], F32, tag="pk",
                                                  name=f"s{l}_{h16}_{jp}_{s}")
                                nc.tensor.matmul(
                                    s_ps[:],
                                    k_sb[off:off + D, hp, s * 128:(s + 1) * 128],
                                    q_sb[off:off + D, hp,
                                         jp * 256:(jp + 1) * 256],
                                    start=True, stop=True)
                                nc.scalar.activation(ex[:, s % 2, :], s_ps[:],
                                                     AF.Exp)
                                if (jp == 0) or (s >= 4):
                                    nc.vector.tensor_tensor(
                                        ex[:, s % 2, :], ex[:, s % 2, :],
                                        mask_sb[:, jp, s - (0 if jp == 0 else 4), :],
                                        OP.mult)
                            if s >= 1:
                                nc.tensor.matmul(
                                    o_ps[:OUT, jp * 256:(jp + 1) * 256],
                                    v_sb[:, s - 1, h16, :],
                                    ex[:, (s - 1) % 2, :],
                                    start=(s == 1), stop=(s == vis))
                    r_sb = sp.tile([1, 512], BF16, tag="rr", name=f"r{l}_{h16}")
                    nc.vector.reciprocal(r_sb[:], o_ps[VOCAB:OUT, :])
                    nc.scalar.copy(o_sb[off:off + D, hp, :], o_ps[:D, :])
                    if pending is not None:
                        _normalize(pending)
                    pending = (h16, r_sb)
                _normalize(pending)

                # ===== Wo + residual =====
                for cot in range(8):
                    wo_sb = w512p.tile([128, 8, 128], BF16, tag="w5",
                                       name=f"wo{l}_{cot}")
                    nc.sync.dma_start(wo_sb[:], wo_ext[l, cot])
                    xo_ps = ps512.tile([128, 512], F32, tag="p5",
                                       name=f"xo{l}{cot}")
                    for hdt in range(8):
                        nc.tensor.matmul(xo_ps[:], wo_sb[:, hdt, :],
                                         o_sb[:, hdt, :],
                                         start=(hdt == 0), stop=(hdt == 7))
                    xo_sb = sp.tile([128, 512], F32, tag="xo", name=f"xs{l}{cot}")
                    nc.scalar.activation(xo_sb[:], xo_ps[:], AF.Identity,
                                         bias=bo_sb[:, l, cot, :])
                    nc.vector.tensor_add(x_sb[:, cot, :], x_sb[:, cot, :],
                                         xo_sb[:])

                # ===== norm2 -> h2 =====
                h2_sb = bigp.tile([128, 8, 512], F32R, tag="big", name=f"h2_{l}")
                rbc2 = rms_rbc(f"b{l}")
                for ct in range(8):
                    nc.vector.tensor_tensor(h2_sb[:, ct, :], x_sb[:, ct, :],
                                            rbc2[:], OP.mult)

                # ===== FFN (ft chunks of 4; W1 f32r, W2 bf16) =====
                for chunk in range(8):
                    u_sb = sp.tile([128, 4, 512], BF16, tag="u",
                                   name=f"u{l}_{chunk}")
                    w2c = []
                    for fi in range(4):
                        ft = chunk * 4 + fi
                        w1_sb = w512p.tile([128, 8, 128], F32R, tag="w5",
                                           name=f"w1_{l}_{ft}")
                        nc.sync.dma_start(w1_sb[:], w1_ext[l, ft])
                        u_ps = ps512.tile([128, 512], F32, tag="p5",
                                          name=f"u{l}{ft}")
                        for ct in range(8):
                            nc.tensor.matmul(u_ps[:], w1_sb[:, ct, :],
                                             h2_sb[:, ct, :],
                                             start=(ct == 0), stop=(ct == 7))
                        nc.scalar.activation(u_sb[:, fi, :], u_ps[:], AF.Gelu,
                                             bias=b1_sb[:, l, ft, :])
                        w2_sb = w2p.tile([128, 8, 128], BF16, tag="w2",
                                         name=f"w2_{l}_{ft}")
                        nc.sync.dma_start(w2_sb[:], w2_ext[l, ft])
                        w2c.append(w2_sb)
                    for cot in range(8):
                        y_ps = ps512.tile([128, 512], F32, tag="p5",
                                          name=f"y{l}{chunk}{cot}")
                        for fi in range(4):
                            nc.tensor.matmul(y_ps[:], w2c[fi][:, cot, :],
                                             u_sb[:, fi, :],
                                             start=(fi == 0), stop=(fi == 3))
                        nc.vector.tensor_add(x_sb[:, cot, :], x_sb[:, cot, :],
                                             y_ps[:])
                for cot in range(8):
                    nc.scalar.add(x_sb[:, cot, :], x_sb[:, cot, :],
                                  b2_sb[:, l, cot, :])

            # ===== lm head + log_softmax / log_sigmoid =====
            for tlt in range(4):
                lg = ps512.tile([128, OUT], F32, tag="p5", name=f"lg{tlt}")
                for ct in range(8):
                    xr = sp.tile([128, 128], BF16, tag="xr", name=f"xr{tlt}_{ct}")
                    nc.scalar.copy(xr[:], x_sb[:, ct, tlt * 128:(tlt + 1) * 128])
                    nc.tensor.matmul(lg[:], xr[:], lmw_sb[:, ct, :],
                                     start=(ct == 0), stop=(ct == 7))
                lgb = sp.tile([128, OUT], F32, tag="lgb", name=f"lgb{tlt}")
                nc.vector.tensor_add(lgb[:], lg[:], lmb_sb[:])
                m = sp.tile([128, 1], F32, tag="m", name=f"m{tlt}")
                nc.vector.reduce_max(m[:], lgb[:, 0:VOCAB], axis=AX.X)
                nm = sp.tile([128, 1], F32, tag="nm", name=f"nm{tlt}")
                nc.scalar.mul(nm[:], m[:], -1.0)
                e = sp.tile([128, VOCAB], F32, tag="e", name=f"e{tlt}")
                es = sp.tile([128, 1], F32, tag="es", name=f"es{tlt}")
                nc.scalar.activation(e[:], lgb[:, 0:VOCAB], AF.Exp, bias=nm[:],
                                     accum_out=es[:])
                lse = sp.tile([128, 1], F32, tag="lse", name=f"lse{tlt}")
                nc.scalar.activation(lse[:], es[:], AF.Ln)
                bt = sp.tile([128, 1], F32, tag="bt", name=f"bt{tlt}")
                nc.vector.tensor_tensor(bt[:], nm[:], lse[:], OP.subtract)
                outt = sp.tile([128, OUT], F32, tag="outt", name=f"ot{tlt}")
                nc.scalar.activation(outt[:, 0:VOCAB], lgb[:, 0:VOCAB],
                                     AF.Identity, bias=bt[:])
                sg = sp.tile([128, 1], F32, tag="sg", name=f"sg{tlt}")
                nc.scalar.activation(sg[:], lgb[:, VOCAB:OUT], AF.Sigmoid)
                nc.scalar.activation(outt[:, VOCAB:OUT], sg[:], AF.Ln)
                nc.sync.dma_start(out_ext[tlt * 128:(tlt + 1) * 128, :], outt[:])

    _split_sync_waits(nc)
    return nc


# ---------------------------------------------------------------------------
# host-side preparation
# ---------------------------------------------------------------------------
def _own_rows(core):
    return np.concatenate(
        [np.arange(b * 128, (b + 1) * 128) for b in OWN_BLOCKS[core % 2]]
    )


def _bf(a):
    return np.asarray(a, dtype=ml_dtypes.bfloat16)


def _f32(a):
    return np.ascontiguousarray(a, dtype=np.float32)


def _prep(inputs):
    acts = np.asarray(inputs["acts"])
    durations = _f32(inputs["durations"])
    emb_table = _f32(inputs["emb_table"])
    pos_table = _f32(inputs["pos_table"])
    Wq, Wk, Wv = (_f32(inputs[k]) for k in ("Wq", "Wk", "Wv"))
    Wo, bo = _f32(inputs["Wo"]), _f32(inputs["bo"])
    W1, b1 = _f32(inputs["W1"]), _f32(inputs["b1"])
    W2, b2 = _f32(inputs["W2"]), _f32(inputs["b2"])
    g1, g2 = _f32(inputs["g1"]), _f32(inputs["g2"])
    lm_W, lm_b = _f32(inputs["lm_W"]), _f32(inputs["lm_b"])

    # fold g1 into Wq/Wk/Wv (q also gets the D^-0.5 score scale), g2 into W1
    Wq_eff = Wq * g1[:, None, :, None] * (D ** -0.5)
    Wk_eff = Wk * g1[:, None, :, None]
    Wv_eff = Wv * g1[:, None, :, None]
    W1_eff = W1 * g2[:, :, None]

    def qk_arr(A):  # [L,H,C,D] -> [L, hp, cp, ct, m]
        A2 = A.transpose(0, 2, 1, 3).reshape(L, C, H * D)
        return _bf(A2.reshape(L, 8, 128, 8, 128).transpose(0, 3, 2, 1, 4))

    shared = {
        "aug_table": None, "ones_col": _bf(np.ones((128, 1))),
        "ones_row": _f32(np.ones((1, 128))),
        "ones_row_bf": _bf(np.ones((1, 128))),
        "Wq_arr": qk_arr(Wq_eff), "Wk_arr": qk_arr(Wk_eff),
        "Wv_arr": _bf(Wv_eff.transpose(0, 2, 1, 3).reshape(L, C, H * D)
                      .reshape(L, 8, 128, 2, 512).transpose(0, 2, 3, 1, 4)),
        "Wo_arr": _bf(Wo.reshape(L, 8, 128, 8, 128).transpose(0, 3, 2, 1, 4)),
        "W1_arr": _f32(W1_eff.reshape(L, 8, 128, 32, 128).transpose(0, 3, 2, 1, 4)),
        "W2_arr": _bf(W2.reshape(L, 32, 128, 8, 128)),
        "bo_fm": bo.reshape(L, 8, 128, 1),
        "b1_fm": b1.reshape(L, 32, 128, 1),
        "b2_fm": b2.reshape(L, 8, 128, 1),
        "lmW_arr": _bf(lm_W.reshape(8, 128, OUT).transpose(1, 0, 2)),
        "lmb_bc": _f32(np.tile(lm_b[None, :], (128, 1))),
    }
    aug = np.zeros((OUT, C), np.float32)
    aug[:VOCAB, : C - 1] = emb_table
    aug[VOCAB, C - 1] = 1.0
    shared["aug_table"] = _f32(aug.reshape(OUT, 8, 128))

    in_maps = []
    for core in range(8):
        b, half = core // 2, core % 2
        rows = _own_rows(core)
        oh = np.zeros((OUT, 512), np.float32)
        oh[acts[b, rows], np.arange(512)] = 1.0
        oh[VOCAB, :] = durations[b, rows]
        pos = pos_table[rows].T.reshape(8, 128, 512).transpose(1, 0, 2)
        masks = np.zeros((2, 4, 128, 256), np.float32)
        for jp in range(2):
            for si in range(4):
                s = si if jp == 0 else 4 + si
                gk = s * 128
                for jh in range(2):  # the two q-tiles inside the pair
                    j = jp * 2 + jh
                    gq = OWN_BLOCKS[half][j] * 128
                    ii = gk + np.arange(128)[:, None]
                    jj = gq + np.arange(128)[None, :]
                    masks[jp, si, :, jh * 128:(jh + 1) * 128] = (
                        ii <= jj).astype(np.float32)
        m = dict(shared)
        m["onehot_t"] = _f32(oh)
        m["pos_fm"] = _bf(pos)
        m["masks"] = _bf(masks)
        in_maps.append(m)
    return in_maps


LAST_EXEC_NS = [None]


def kernel(**inputs) -> np.ndarray:
    nc = build_graph()
    in_maps = _prep(inputs)
    trace = bool(int(os.environ.get("KERNEL_TRACE", "0")))
    res = bass_utils.run_bass_kernel_spmd(
        nc, in_maps, list(range(8)), trace=trace,
        trace_cores=[0] if trace else None,
    )
    LAST_EXEC_NS[0] = res.exec_time_ns
    if trace and res.instructions_and_trace:
        print("trace path:", res.instructions_and_trace[1])
    full = np.zeros((B, T, OUT), np.float32)
    for core in range(8):
        full[core // 2, _own_rows(core)] = res.results[core]["out"]
    return full


# revision 17
# speedup vs baseline: 1.2975x; 1.0147x over previous
"""Trainium2 Bass kernel for nn_AttentionDecoder_82738249990894 (B=4, T=1024,
C=1024, H=16, D=64, F=4096, L=4, vocab 64+1 outputs).

Sharding: sequence-split data parallel over 8 cores.  Core c handles batch
b = c//2, sequence half = c%2.  Balanced causal split: half0 owns global
128-row blocks [0,1,6,7], half1 owns [2,3,4,5] (equal attention work: both
see 18 causal k-tiles).  Per layer the pair exchanges rmsnorm'd activations
(bf16, pairwise AllGather, ~1MB) and each core recomputes k/v for all 1024
tokens locally.  No other communication.

SPMD uniformity: one graph runs on all 8 cores, so the key/value strip is
kept in GLOBAL token order (the AllGather return scatters both pair slots
to fixed global positions) and every local q-tile j computes scores against
the union visibility vis_u=[3,4,7,8] k-tiles; per-core 0/1 masks (input
data) encode causality and half-dependent visibility.

Matmul dtypes: bf16 on the attention path (h, Wq/Wk/Wv, q/k/v, softmax
weights) and for W2; float32r (4-byte, full TensorE rate) for Wo/W1/lm.
Residual x stays fp32.  Softmax skips max-subtraction (scores are O(10);
fp32 psum exp is safe) and gets denominators free via a ones-column
appended to v; normalization is deferred to after the AV matmul.
"""
import os
import sys
import types

sys.path.insert(0, "/opt/trn_rl_repo")

import numpy as np
import ml_dtypes

import antenv

if not hasattr(antenv, "axon_hooks"):
    _mod = types.ModuleType("antenv.axon_hooks")
    _mod._hook = None
    _mod.set_axon_ntff_profile_hook = lambda h: setattr(_mod, "_hook", h)
    _mod.get_axon_ntff_profile_hook = lambda: _mod._hook
    sys.modules["antenv.axon_hooks"] = _mod
    antenv.axon_hooks = _mod
    try:
        from trn_agent_boot.trn_boot import _ntff_profile_via_ctypes

        _mod.set_axon_ntff_profile_hook(
            _ntff_profile_via_ctypes("/opt/axon/libaxon_pjrt.so")
        )
    except Exception:
        pass

import concourse.bass as bass
import concourse.mybir as mybir
import concourse.tile as tile
from concourse import bass_utils

bass_utils.upload_artifacts = lambda tmpdir: "local://" + tmpdir
try:
    from concourse import tile_utils as _tu

    _tu.max_sbuf_usage = 206 * 1024
except Exception:
    pass

F32 = mybir.dt.float32
F32R = mybir.dt.float32r
BF16 = mybir.dt.bfloat16
AF = mybir.ActivationFunctionType
OP = mybir.AluOpType
AX = mybir.AxisListType

B, T, C, H, D, F, L = 4, 1024, 1024, 16, 64, 4096, 4
VOCAB, OUT = 64, 65
EPS = float(np.finfo(np.float32).eps)
RG = [[0, 1], [2, 3], [4, 5], [6, 7]]
OWN_BLOCKS = {0: [0, 1, 6, 7], 1: [2, 3, 4, 5]}
VIS_U = [3, 4, 7, 8]          # union visible k-tiles per local q-tile
N_MASK = 3                    # last 3 visible slots carry a mask

_wsplit_ctr = [0]


def _split_sync_waits(nc):
    """This walrus build allows one sync-wait per instruction; hoist extras
    onto injected same-engine NoOps."""
    for f in nc.m.functions:
        for bb in f.blocks:
            out = []
            changed = False
            for inst in bb.instructions:
                si = getattr(inst, "sync_info", None)
                if si is not None and si.on_wait is not None and len(si.on_wait) > 1:
                    waits = list(si.on_wait)
                    for w in waits[:-1]:
                        _wsplit_ctr[0] += 1
                        n = mybir.InstNoOp(
                            name=f"WSPLIT-{_wsplit_ctr[0]}", ins=[], outs=[]
                        )
                        n.engine = inst.engine
                        n.sync_info = mybir.SyncInfo(on_wait=[w], on_update=[])
                        out.append(n)
                    inst.sync_info = mybir.SyncInfo(
                        on_wait=[waits[-1]], on_update=list(si.on_update)
                    )
                    changed = True
                out.append(inst)
            if changed:
                bb.instructions[:] = out


def build_graph():
    nc = bass.Bass()
    dp = nc.declare_dram_parameter
    onehot_ext = dp("onehot_t", [OUT, 512], F32R, isOutput=False)
    pos_ext = dp("pos_fm", [128, 8, 512], BF16, isOutput=False)
    aug_ext = dp("aug_table", [OUT, 8, 128], F32R, isOutput=False)
    mask_ext = dp("masks", [2, 4, 128, 256], BF16, isOutput=False)
    onescol_ext = dp("ones_col", [128, 1], BF16, isOutput=False)
    onesrow_ext = dp("ones_row", [1, 128], F32R, isOutput=False)
    onesrowb_ext = dp("ones_row_bf", [1, 128], BF16, isOutput=False)
    wq_ext = dp("Wq_arr", [L, 8, 128, 8, 128], BF16, isOutput=False)
    wk_ext = dp("Wk_arr", [L, 8, 128, 8, 128], BF16, isOutput=False)
    wv_ext = dp("Wv_arr", [L, 128, 2, 8, 512], BF16, isOutput=False)
    wo_ext = dp("Wo_arr", [L, 8, 128, 8, 128], BF16, isOutput=False)
    w1_ext = dp("W1_arr", [L, 32, 128, 8, 128], F32R, isOutput=False)
    w2_ext = dp("W2_arr", [L, 32, 128, 8, 128], BF16, isOutput=False)
    bo_ext = dp("bo_fm", [L, 8, 128, 1], F32, isOutput=False)
    b1_ext = dp("b1_fm", [L, 32, 128, 1], F32, isOutput=False)
    b2_ext = dp("b2_fm", [L, 8, 128, 1], F32, isOutput=False)
    lmw_ext = dp("lmW_arr", [128, 8, OUT], BF16, isOutput=False)
    lmb_ext = dp("lmb_bc", [128, OUT], F32, isOutput=False)
    out_ext = dp("out", [512, OUT], F32, isOutput=True)

    with tile.TileContext(nc) as tc:
        nc_lp = nc.allow_low_precision(reason="bf16 attention path is intentional")
        nc_lp.__enter__()
        with (
            tc.tile_pool(name="persist", bufs=1) as pp,
            tc.tile_pool(name="scratch", bufs=2) as sp,
            tc.tile_pool(name="wqk", bufs=3) as wqkp,
            tc.tile_pool(name="w512", bufs=3) as w512p,
            tc.tile_pool(name="w2p", bufs=5) as w2p,
            tc.tile_pool(name="bigp", bufs=1) as bigp,
            tc.tile_pool(name="wvp", bufs=1) as wvp,
            tc.tile_pool(name="ps512", bufs=3, space="PSUM") as ps512,
            tc.tile_pool(name="ps128", bufs=3, space="PSUM") as ps128,
            tc.tile_pool(name="ps128o", bufs=2, space="PSUM") as ps128o,
            tc.tile_pool(name="dram", bufs=2, space="DRAM") as dram,
        ):
            # ---- constants ----
            ones_col = pp.tile([128, 1], BF16)
            ones_row = pp.tile([1, 128], F32R)
            ones_row_bf = pp.tile([1, 128], BF16)
            aug_sb = pp.tile([OUT, 8, 128], F32R)
            onehot_sb = pp.tile([OUT, 512], F32R)
            mask_sb = pp.tile([128, 2, 4, 256], BF16)
            lmw_sb = pp.tile([128, 8, OUT], BF16)
            lmb_sb = pp.tile([128, OUT], F32)
            bo_sb = pp.tile([128, L, 8, 1], F32)
            b1_sb = pp.tile([128, L, 32, 1], F32)
            b2_sb = pp.tile([128, L, 8, 1], F32)
            nc.sync.dma_start(ones_col[:], onescol_ext[:])
            nc.sync.dma_start(ones_row[:], onesrow_ext[:])
            nc.sync.dma_start(ones_row_bf[:], onesrowb_ext[:])
            nc.sync.dma_start(aug_sb[:], aug_ext[:])
            nc.sync.dma_start(onehot_sb[:], onehot_ext[:])
            nc.sync.dma_start(mask_sb[:], mask_ext.rearrange("j s p m -> p j s m"))
            nc.sync.dma_start(lmw_sb[:], lmw_ext[:])
            nc.sync.dma_start(lmb_sb[:], lmb_ext[:])
            nc.sync.dma_start(bo_sb[:], bo_ext.rearrange("l t p o -> p l t o"))
            nc.sync.dma_start(b1_sb[:], b1_ext.rearrange("l t p o -> p l t o"))
            nc.sync.dma_start(b2_sb[:], b2_ext.rearrange("l t p o -> p l t o"))

            eps_sb = pp.tile([128, 1], F32)
            nc.gpsimd.memset(eps_sb[:], EPS)

            # ---- persistent activations ----
            x_sb = pp.tile([128, 8, 512], F32)       # residual (feature-major)
            h_own = pp.tile([128, 8, 512], BF16)     # norm'd own tokens
            h_str = pp.tile([128, 8, 1024], BF16)    # norm'd pair, global order
            q_sb = pp.tile([128, 8, 512], BF16)      # [2h*64, hp, local t]
            k_sb = pp.tile([128, 8, 1024], BF16)     # [2h*64, hp, global t]
            v_sb = pp.tile([128, 8, 16, OUT], BF16)  # [tk, tkt, head, d+1]
            o_sb = pp.tile([128, 8, 512], BF16)      # attn out [hd, hdt, local]
            # pos and per-layer h2 share one big slot (disjoint lifetimes)
            pos_sb = bigp.tile([128, 8, 512], BF16, tag="big", name="pos")
            nc.sync.dma_start(pos_sb[:], pos_ext[:])

            # ---- embedding: x = onehot @ aug_table + pos ----
            for ct in range(8):
                emb_ps = ps512.tile([128, 512], F32, tag="p5", name=f"emb{ct}")
                nc.tensor.matmul(emb_ps[:], aug_sb[:, ct, :], onehot_sb[:],
                                 start=True, stop=True)
                nc.vector.tensor_add(x_sb[:, ct, :], emb_ps[:], pos_sb[:, ct, :])

            def rms_rbc(tag):
                ssum = ps512.tile([128, 512], F32, tag="p5", name=f"ss{tag}")
                for ct in range(8):
                    xsq = sp.tile([128, 512], BF16, tag="xsq", name=f"xq{tag}{ct}")
                    nc.scalar.activation(xsq[:], x_sb[:, ct, :], AF.Square)
                    nc.tensor.matmul(ssum[:1, :], ones_col[:], xsq[:],
                                     start=(ct == 0), stop=(ct == 7))
                sqv2 = sp.tile([1, 512], F32R, tag="sqv", name=f"sv{tag}",
                               bufs=1)
                nc.scalar.activation(sqv2[:], ssum[:1, :], AF.Sqrt,
                                     bias=eps_sb[:1, :], scale=1.0 / C)
                rbc = ps512.tile([128, 512], F32, tag="p5", name=f"rb{tag}")
                nc.tensor.matmul(rbc[:], ones_row[:], sqv2[:], start=True,
                                 stop=True)
                rinv = sp.tile([128, 512], F32R, tag="rinv", name=f"ri{tag}")
                nc.vector.reciprocal(rinv[:], rbc[:])
                return rinv

            for l in range(L):
                # ===== norm1 -> h_own =====
                rbc = rms_rbc(f"a{l}")
                for ct in range(8):
                    nc.vector.tensor_tensor(h_own[:, ct, :], x_sb[:, ct, :],
                                            rbc[:], OP.mult)

                # ===== pair exchange (AllGather) =====
                bounce = dram.tile([8, 128, 512], BF16, tag="agin", name=f"agi{l}")
                for ct in range(8):
                    nc.sync.dma_start(bounce[ct], h_own[:, ct, :])
                gath = dram.tile([2, 8, 128, 512], BF16, tag="agout",
                                 name=f"ago{l}")
                nc.gpsimd.collective_compute(
                    "AllGather", OP.bypass,
                    ins=[bounce[:].opt()],
                    outs=[gath[:].opt()],
                    replica_groups=RG,
                )

                # Wv for this layer (no AG dependency -> overlaps exchange)
                wv_sb = wvp.tile([128, 2, 8, 512], BF16, tag="wv", name=f"wv{l}")
                nc.sync.dma_start(wv_sb[:], wv_ext[l])

                # ===== q from h_own (overlaps AG) =====
                for hp in range(8):
                    wq_sb = wqkp.tile([128, 8, 128], BF16, tag="wqk",
                                      name=f"wq{l}_{hp}")
                    nc.sync.dma_start(wq_sb[:], wq_ext[l, hp])
                    q_ps = ps512.tile([128, 512], F32, tag="p5", name=f"q{l}{hp}")
                    for ct in range(8):
                        nc.tensor.matmul(q_ps[:], wq_sb[:, ct, :],
                                         h_own[:, ct, :],
                                         start=(ct == 0), stop=(ct == 7))
                    nc.scalar.copy(q_sb[:, hp, :], q_ps[:])

                # ===== scatter AG result into global-order strip =====
                # slot0 = half0 local blocks -> global [0,1,6,7]
                # slot1 = half1 local blocks -> global [2,3,4,5]
                for ct in range(8):
                    nc.sync.dma_start(h_str[:, ct, 0:256], gath[0, ct, :, 0:256])
                    nc.sync.dma_start(h_str[:, ct, 768:1024], gath[0, ct, :, 256:512])
                    nc.sync.dma_start(h_str[:, ct, 256:768], gath[1, ct])

                # ===== k over the strip =====
                for hp in range(8):
                    wk_sb = wqkp.tile([128, 8, 128], BF16, tag="wqk",
                                      name=f"wk{l}_{hp}")
                    nc.sync.dma_start(wk_sb[:], wk_ext[l, hp])
                    for half in range(2):
                        k_ps = ps512.tile([128, 512], F32, tag="p5",
                                          name=f"k{l}{hp}{half}")
                        for ct in range(8):
                            nc.tensor.matmul(
                                k_ps[:], wk_sb[:, ct, :],
                                h_str[:, ct, half * 512:(half + 1) * 512],
                                start=(ct == 0), stop=(ct == 7))
                        nc.scalar.copy(
                            k_sb[:, hp, half * 512:(half + 1) * 512], k_ps[:])

                # ===== v over the strip (token-major, +ones column) =====
                for tkt in range(8):
                    for vh in range(2):
                        v_ps = ps512.tile([128, 512], F32, tag="p5",
                                          name=f"v{l}{tkt}{vh}")
                        for ct in range(8):
                            nc.tensor.matmul(
                                v_ps[:], h_str[:, ct, tkt * 128:(tkt + 1) * 128],
                                wv_sb[:, vh, ct, :],
                                start=(ct == 0), stop=(ct == 7))
                        nc.scalar.copy(
                            v_sb[:, tkt, 8 * vh:8 * vh + 8, 0:D],
                            v_ps[:].rearrange("p (q d) -> p q d", d=D))
                    nc.gpsimd.memset(v_sb[:, tkt, :, D:OUT], 1.0)

                # ===== attention (j-pair batched, deferred softmax
                # normalization: denominators collected per head, one batched
                # reciprocal per layer, PE broadcast via eye_sel) =====
                VIS_P = [4, 8]
                pending = None

                def _normalize(pend):
                    ph, pden = pend
                    php, poff = ph // 2, (ph % 2) * D
                    rb_ps = ps128.tile([128, 512], F32, tag="pk",
                                       name=f"rb{l}_{ph}")
                    nc.tensor.matmul(rb_ps[poff:poff + D, :],
                                     ones_row_bf[:, 0:D],
                                     pden[:], start=True, stop=True)
                    rinv = sp.tile([128, 512], BF16, tag="rina",
                                   name=f"rn{l}_{ph}")
                    nc.vector.reciprocal(rinv[poff:poff + D, :],
                                         rb_ps[poff:poff + D, :])
                    nc.vector.tensor_tensor(
                        o_sb[poff:poff + D, php, :], o_sb[poff:poff + D, php, :],
                        rinv[poff:poff + D, :], OP.mult)

                for h16 in range(16):
                    hp, off = h16 // 2, (h16 % 2) * D
                    o_ps = ps128o.tile([128, 512], F32, tag="po",
                                       name=f"o{l}_{h16}")
                    for jp in range(2):
                        vis = VIS_P[jp]
                        ex = sp.tile([128, 3, 256], BF16, tag="exp",
                                     name=f"ex{l}_{h16}_{jp}")
                        for s in range(vis + 2):
                            if s < vis:
                                s_ps = ps128.tile([128, 256], F32, tag="pk",
                                                  name=f"s{l}_{h16}_{jp}_{s}")
                                nc.tensor.matmul(
                                    s_ps[:],
                                    k_sb[off:off + D, hp, s * 128:(s + 1) * 128],
                                    q_sb[off:off + D, hp,
                                         jp * 256:(jp + 1) * 256],
                                    start=True, stop=True)
                                nc.scalar.activation(ex[:, s % 3, :], s_ps[:],
                                                     AF.Exp)
                                if (jp == 0) or (s >= 4):
                                    nc.vector.tensor_tensor(
                                        ex[:, s % 3, :], ex[:, s % 3, :],
                                        mask_sb[:, jp, s - (0 if jp == 0 else 4), :],
                                        OP.mult)
                            if s >= 2:
                                nc.tensor.matmul(
                                    o_ps[:OUT, jp * 256:(jp + 1) * 256],
                                    v_sb[:, s - 2, h16, :],
                                    ex[:, (s - 2) % 3, :],
                                    start=(s == 2), stop=(s == vis + 1))
                    den = sp.tile([1, 512], BF16, tag="rr", name=f"r{l}_{h16}")
                    nc.scalar.copy(den[:], o_ps[VOCAB:OUT, :])
                    nc.scalar.copy(o_sb[off:off + D, hp, :], o_ps[:D, :])
                    if pending is not None:
                        _normalize(pending)
                    pending = (h16, den)
                _normalize(pending)

                # ===== Wo + residual =====
                for cot in range(8):
                    wo_sb = w512p.tile([128, 8, 128], BF16, tag="w5",
                                       name=f"wo{l}_{cot}")
                    nc.sync.dma_start(wo_sb[:], wo_ext[l, cot])
                    xo_ps = ps512.tile([128, 512], F32, tag="p5",
                                       name=f"xo{l}{cot}")
                    for hdt in range(8):
                        nc.tensor.matmul(xo_ps[:], wo_sb[:, hdt, :],
                                         o_sb[:, hdt, :],
                                         start=(hdt == 0), stop=(hdt == 7))
                    xo_sb = sp.tile([128, 512], F32, tag="xo", name=f"xs{l}{cot}")
                    nc.scalar.activation(xo_sb[:], xo_ps[:], AF.Identity,
                                         bias=bo_sb[:, l, cot, :])
                    nc.vector.tensor_add(x_sb[:, cot, :], x_sb[:, cot, :],
                                         xo_sb[:])

                # ===== norm2 -> h2 =====
                h2_sb = bigp.tile([128, 8, 512], F32R, tag="big", name=f"h2_{l}")
                rbc2 = rms_rbc(f"b{l}")
                for ct in range(8):
                    nc.vector.tensor_tensor(h2_sb[:, ct, :], x_sb[:, ct, :],
                                            rbc2[:], OP.mult)

                # ===== FFN (ft chunks of 4; W1 f32r, W2 bf16) =====
                for chunk in range(8):
                    u_sb = sp.tile([128, 4, 512], BF16, tag="u",
                                   name=f"u{l}_{chunk}")
                    w2c = []
                    for fi in range(4):
                        ft = chunk * 4 + fi
                        w1_sb = w512p.tile([128, 8, 128], F32R, tag="w5",
                                           name=f"w1_{l}_{ft}")
                        nc.sync.dma_start(w1_sb[:], w1_ext[l, ft])
                        u_ps = ps512.tile([128, 512], F32, tag="p5",
                                          name=f"u{l}{ft}")
                        for ct in range(8):
                            nc.tensor.matmul(u_ps[:], w1_sb[:, ct, :],
                                             h2_sb[:, ct, :],
                                             start=(ct == 0), stop=(ct == 7))
                        nc.scalar.activation(u_sb[:, fi, :], u_ps[:], AF.Gelu,
                                             bias=b1_sb[:, l, ft, :])
                        w2_sb = w2p.tile([128, 8, 128], BF16, tag="w2",
                                         name=f"w2_{l}_{ft}")
                        nc.sync.dma_start(w2_sb[:], w2_ext[l, ft])
                        w2c.append(w2_sb)
                    for cot in range(8):
                        y_ps = ps512.tile([128, 512], F32, tag="p5",
                                          name=f"y{l}{chunk}{cot}")
                        for fi in range(4):
                            nc.tensor.matmul(y_ps[:], w2c[fi][:, cot, :],
                                             u_sb[:, fi, :],
                                             start=(fi == 0), stop=(fi == 3))
                        nc.vector.tensor_add(x_sb[:, cot, :], x_sb[:, cot, :],
                                             y_ps[:])
                for cot in range(8):
                    nc.scalar.add(x_sb[:, cot, :], x_sb[:, cot, :],
                                  b2_sb[:, l, cot, :])

            # ===== lm head + log_softmax / log_sigmoid =====
            for tlt in range(4):
                lg = ps512.tile([128, OUT], F32, tag="p5", name=f"lg{tlt}")
                for ct in range(8):
                    xr = sp.tile([128, 128], BF16, tag="xr", name=f"xr{tlt}_{ct}")
                    nc.scalar.copy(xr[:], x_sb[:, ct, tlt * 128:(tlt + 1) * 128])
                    nc.tensor.matmul(lg[:], xr[:], lmw_sb[:, ct, :],
                                     start=(ct == 0), stop=(ct == 7))
                lgb = sp.tile([128, OUT], F32, tag="lgb", name=f"lgb{tlt}")
                nc.vector.tensor_add(lgb[:], lg[:], lmb_sb[:])
                m = sp.tile([128, 1], F32, tag="m", name=f"m{tlt}")
                nc.vector.reduce_max(m[:], lgb[:, 0:VOCAB], axis=AX.X)
                nm = sp.tile([128, 1], F32, tag="nm", name=f"nm{tlt}")
                nc.scalar.mul(nm[:], m[:], -1.0)
                e = sp.tile([128, VOCAB], F32, tag="e", name=f"e{tlt}")
                es = sp.tile([128, 1], F32, tag="es", name=f"es{tlt}")
                nc.scalar.activation(e[:], lgb[:, 0:VOCAB], AF.Exp, bias=nm[:],
                                     accum_out=es[:])
                lse = sp.tile([128, 1], F32, tag="lse", name=f"lse{tlt}")
                nc.scalar.activation(lse[:], es[:], AF.Ln)
                bt = sp.tile([128, 1], F32, tag="bt", name=f"bt{tlt}")
                nc.vector.tensor_tensor(bt[:], nm[:], lse[:], OP.subtract)
                outt = sp.tile([128, OUT], F32, tag="outt", name=f"ot{tlt}")
                nc.scalar.activation(outt[:, 0:VOCAB], lgb[:, 0:VOCAB],
                                     AF.Identity, bias=bt[:])
                sg = sp.tile([128, 1], F32, tag="sg", name=f"sg{tlt}")
                nc.scalar.activation(sg[:], lgb[:, VOCAB:OUT], AF.Sigmoid)
                nc.scalar.activation(outt[:, VOCAB:OUT], sg[:], AF.Ln)
                nc.sync.dma_start(out_ext[tlt * 128:(tlt + 1) * 128, :], outt[:])

    _split_sync_waits(nc)
    return nc


# ---------------------------------------------------------------------------
# host-side preparation
# ---------------------------------------------------------------------------
def _own_rows(core):
    return np.concatenate(
        [np.arange(b * 128, (b + 1) * 128) for b in OWN_BLOCKS[core % 2]]
    )


def _bf(a):
    return np.asarray(a, dtype=ml_dtypes.bfloat16)


def _f32(a):
    return np.ascontiguousarray(a, dtype=np.float32)


def _prep(inputs):
    acts = np.asarray(inputs["acts"])
    durations = _f32(inputs["durations"])
    emb_table = _f32(inputs["emb_table"])
    pos_table = _f32(inputs["pos_table"])
    Wq, Wk, Wv = (_f32(inputs[k]) for k in ("Wq", "Wk", "Wv"))
    Wo, bo = _f32(inputs["Wo"]), _f32(inputs["bo"])
    W1, b1 = _f32(inputs["W1"]), _f32(inputs["b1"])
    W2, b2 = _f32(inputs["W2"]), _f32(inputs["b2"])
    g1, g2 = _f32(inputs["g1"]), _f32(inputs["g2"])
    lm_W, lm_b = _f32(inputs["lm_W"]), _f32(inputs["lm_b"])

    # fold g1 into Wq/Wk/Wv (q also gets the D^-0.5 score scale), g2 into W1
    Wq_eff = Wq * g1[:, None, :, None] * (D ** -0.5)
    Wk_eff = Wk * g1[:, None, :, None]
    Wv_eff = Wv * g1[:, None, :, None]
    W1_eff = W1 * g2[:, :, None]

    def qk_arr(A):  # [L,H,C,D] -> [L, hp, cp, ct, m]
        A2 = A.transpose(0, 2, 1, 3).reshape(L, C, H * D)
        return _bf(A2.reshape(L, 8, 128, 8, 128).transpose(0, 3, 2, 1, 4))

    shared = {
        "aug_table": None, "ones_col": _bf(np.ones((128, 1))),
        "ones_row": _f32(np.ones((1, 128))),
        "ones_row_bf": _bf(np.ones((1, 128))),
        "Wq_arr": qk_arr(Wq_eff), "Wk_arr": qk_arr(Wk_eff),
        "Wv_arr": _bf(Wv_eff.transpose(0, 2, 1, 3).reshape(L, C, H * D)
                      .reshape(L, 8, 128, 2, 512).transpose(0, 2, 3, 1, 4)),
        "Wo_arr": _bf(Wo.reshape(L, 8, 128, 8, 128).transpose(0, 3, 2, 1, 4)),
        "W1_arr": _f32(W1_eff.reshape(L, 8, 128, 32, 128).transpose(0, 3, 2, 1, 4)),
        "W2_arr": _bf(W2.reshape(L, 32, 128, 8, 128)),
        "bo_fm": bo.reshape(L, 8, 128, 1),
        "b1_fm": b1.reshape(L, 32, 128, 1),
        "b2_fm": b2.reshape(L, 8, 128, 1),
        "lmW_arr": _bf(lm_W.reshape(8, 128, OUT).transpose(1, 0, 2)),
        "lmb_bc": _f32(np.tile(lm_b[None, :], (128, 1))),
    }
    aug = np.zeros((OUT, C), np.float32)
    aug[:VOCAB, : C - 1] = emb_table
    aug[VOCAB, C - 1] = 1.0
    shared["aug_table"] = _f32(aug.reshape(OUT, 8, 128))

    in_maps = []
    for core in range(8):
        b, half = core // 2, core % 2
        rows = _own_rows(core)
        oh = np.zeros((OUT, 512), np.float32)
        oh[acts[b, rows], np.arange(512)] = 1.0
        oh[VOCAB, :] = durations[b, rows]
        pos = pos_table[rows].T.reshape(8, 128, 512).transpose(1, 0, 2)
        masks = np.zeros((2, 4, 128, 256), np.float32)
        for jp in range(2):
            for si in range(4):
                s = si if jp == 0 else 4 + si
                gk = s * 128
                for jh in range(2):  # the two q-tiles inside the pair
                    j = jp * 2 + jh
                    gq = OWN_BLOCKS[half][j] * 128
                    ii = gk + np.arange(128)[:, None]
                    jj = gq + np.arange(128)[None, :]
                    masks[jp, si, :, jh * 128:(jh + 1) * 128] = (
                        ii <= jj).astype(np.float32)
        m = dict(shared)
        m["onehot_t"] = _f32(oh)
        m["pos_fm"] = _bf(pos)
        m["masks"] = _bf(masks)
        in_maps.append(m)
    return in_maps


LAST_EXEC_NS = [None]


def kernel(**inputs) -> np.ndarray:
    nc = build_graph()
    in_maps = _prep(inputs)
    trace = bool(int(os.environ.get("KERNEL_TRACE", "0")))
    res = bass_utils.run_bass_kernel_spmd(
        nc, in_maps, list(range(8)), trace=trace,
        trace_cores=[0] if trace else None,
    )
    LAST_EXEC_NS[0] = res.exec_time_ns
    if trace and res.instructions_and_trace:
        print("trace path:", res.instructions_and_trace[1])
    full = np.zeros((B, T, OUT), np.float32)
    for core in range(8):
        full[core // 2, _own_rows(core)] = res.results[core]["out"]
    return full
